# revision 18
# baseline (speedup 1.0000x reference)
"""Trainium2 Bass kernel: fused attention block (QKV proj -> MHA -> out proj).

Reference (per batch item b, NUM_HEADS=12, Dh=64):
    qkv = x @ W_qkv; q,k,v per head
    attn = softmax(q @ k^T / 8) @ v
    out  = concat_heads(attn) @ W_proj + b_proj

Sharding: data-parallel over batch across 8 NeuronCores (128 batch items
per core), weights replicated. One SPMD Bass program, per-core inputs.

Design (bf16, feature-major I/O, half-array head pairing, fully
software-pipelined):
  - Host pre-transposes x to feature-major [C, TOK] bf16 and transposes
    the feature-major bf16 output back; device never transposes.
  - Groups of G=8 batches (T=392 tokens). Per group:
    B: q,k co-tiles [128, T] feature-major = Wqkv_slice.T @ xT.
    C: v token-major in 4 tiles of 98 tokens, scattered by SBUF->SBUF
       DMA into per-(head-pair j, batch b) blocks vbd2 [128, 64]:
       rows 0:49 = even-head key positions, 64:113 = odd-head.
    D: per j: 8 batches x 2 concurrent half-array matmuls (even head in
       array quadrants (0:64)x(0:64), odd head at tile_position=(64,64))
       for scores and attn@V; single-op exp on ACT; row sums via ones
       matmul; normalization via reciprocal_approx_fast + broadcast
       matmul + DVE multiply.
    E: out co-tiles [128, T] = Wproj_slice.T @ unT + per-partition bias.
  - Steady state runs D in head-pair double-steps and interleaves E of
    the previous group plus B and C of the next group into them, so the
    tensor engine sees one dense instruction stream and the HAM clock
    gate stays at full rate.
"""
import sys

sys.path.insert(0, "/opt/trn_rl_repo")

import numpy as np
import ml_dtypes

NUM_CORES = 8
B_CORE = 128          # batch items per core
SEQ = 49              # tokens per batch item
C = 768               # channels
H = 12                # heads
G = 8                 # batch items per group
T = SEQ * G           # 392 tokens per group
TOK = B_CORE * SEQ    # 6272 tokens per core
N_GROUPS = B_CORE // G

BF = ml_dtypes.bfloat16

_CACHE = {}


def _consts():
    onesbd = np.zeros((128, 2), dtype=BF)
    onesbd[0:49, 0] = 1.0
    onesbd[64:113, 1] = 1.0
    selpair = np.zeros((2, 128), dtype=BF)
    selpair[0, 0:64] = 1.0
    selpair[1, 64:128] = 1.0
    return {"onesbd": onesbd, "selpair": selpair}


def _build():
    import concourse.bacc as bacc
    import concourse.mybir as mybir
    import concourse.tile as tile

    F32 = mybir.dt.float32
    BF16 = mybir.dt.bfloat16
    EXP = mybir.ActivationFunctionType.Exp

    nc = bacc.Bacc("TRN2", target_bir_lowering=False)

    d_x = nc.declare_dram_parameter("x", [C, TOK], BF16, isOutput=False)
    d_wqkv = nc.declare_dram_parameter("wqkv", [C, 3 * C], BF16, isOutput=False)
    d_wproj = nc.declare_dram_parameter("wproj", [C, C], BF16, isOutput=False)
    d_bias = nc.declare_dram_parameter("bias", [128, 6], F32, isOutput=False)
    d_onesbd = nc.declare_dram_parameter("onesbd", [128, 2], BF16,
                                         isOutput=False)
    d_selpair = nc.declare_dram_parameter("selpair", [2, 128], BF16,
                                          isOutput=False)
    d_out = nc.declare_dram_parameter("out", [C, TOK], BF16, isOutput=True)

    # DRAM views with the 6x128 channel-tile structure exposed, so one DMA
    # moves all 6 channel tiles of a slice
    x6 = d_x.rearrange("(c p) t -> p c t", p=128)
    wqkv6 = d_wqkv.rearrange("(c p) n -> p c n", p=128)
    wproj6 = d_wproj.rearrange("(c p) n -> p c n", p=128)

    with tile.TileContext(nc) as tc, \
         nc.allow_low_precision(reason="bf16 matmuls within 2e-2 tolerance"):
        with tc.tile_pool(name="wres", bufs=1) as wres, \
             tc.tile_pool(name="xT", bufs=2) as p_xT, \
             tc.tile_pool(name="qk", bufs=3) as p_qk, \
             tc.tile_pool(name="scr", bufs=3) as p_scr, \
             tc.tile_pool(name="vbd2", bufs=2) as p_vbd2, \
             tc.tile_pool(name="rr", bufs=3) as p_rr, \
             tc.tile_pool(name="unT", bufs=3) as p_unT, \
             tc.tile_pool(name="osb", bufs=2) as p_osb, \
             tc.tile_pool(name="psA", bufs=2, space="PSUM") as psA, \
             tc.tile_pool(name="psS", bufs=2, space="PSUM") as psS, \
             tc.tile_pool(name="psO", bufs=2, space="PSUM") as psO, \
             tc.tile_pool(name="psV", bufs=2, space="PSUM") as psV:

            # ---- resident weights / constants ----
            # q,k weight columns first: the first B-stage matmuls need
            # only these plus the group-0 x tiles
            w_qkv = wres.tile([128, 6 * 3 * C], BF16, tag="wqkv", name="wqkv")
            wq6 = w_qkv.rearrange("p (c n) -> p c n", n=3 * C)
            # chunked so the startup transfers spread across DMA queues
            for o in range(0, 1536, 384):
                nc.sync.dma_start(wq6[:, :, o:o + 384],
                                  wqkv6[:, :, o:o + 384])

            S = [dict() for _ in range(N_GROUPS)]

            def load_xT(g, chunks=1):
                t = p_xT.tile([128, 6 * T], BF16, tag="xT", name="xT")
                tv = t.rearrange("p (c t) -> p c t", t=T)
                step = T // chunks
                for o in range(0, T, step):
                    nc.sync.dma_start(
                        tv[:, :, o:o + step],
                        x6[:, :, g * T + o:g * T + o + step])
                S[g]["xT"] = tv

            load_xT(0, chunks=4)
            for o in range(1536, 2304, 384):
                nc.sync.dma_start(wq6[:, :, o:o + 384],
                                  wqkv6[:, :, o:o + 384])
            w_proj = wres.tile([128, 6 * C], BF16, tag="wproj", name="wproj")
            wp6 = w_proj.rearrange("p (c n) -> p c n", n=C)
            for o in range(0, C, 384):
                nc.sync.dma_start(wp6[:, :, o:o + 384],
                                  wproj6[:, :, o:o + 384])
            onesbd = wres.tile([128, 2], BF16, tag="onesbd")
            nc.sync.dma_start(onesbd[:], d_onesbd[:])
            selpair = wres.tile([2, 128], BF16, tag="selpair")
            nc.sync.dma_start(selpair[:], d_selpair[:])
            bias_sb = wres.tile([128, 6], F32, tag="bias_sb")
            nc.sync.dma_start(bias_sb[:], d_bias[:])
            # exp tiles: dead bands (rows 49:64, 113:128) must stay zero
            eTs = []
            for nm in ("eTa", "eTb"):
                t = wres.tile([128, T], BF16, tag=nm, name=nm)
                nc.vector.memset(t[:], 0.0)
                eTs.append(t)

            # ---- emission helpers ----
            def emit_B_chain(g, jc):
                st = S[g]
                if jc == 0:
                    st["q"], st["k"] = [None] * 6, [None] * 6
                xT = st["xT"]
                pq = psA.tile([128, T], F32, tag="psA", name="psA")
                for ci in range(6):
                    nc.tensor.matmul(
                        pq[:], wq6[:, ci, 128 * jc:128 * (jc + 1)],
                        xT[:, ci, :], start=(ci == 0), stop=(ci == 5))
                nm = f"q{jc}" if jc < 6 else f"k{jc - 6}"
                t = p_qk.tile([128, T], BF16, tag=nm, name=nm)
                if jc % 2 == 0:
                    nc.scalar.copy(t[:], pq[:])
                else:
                    nc.vector.tensor_copy(t[:], pq[:])
                if jc < 6:
                    st["q"][jc] = t
                else:
                    st["k"][jc - 6] = t

            def emit_C_unit(g, tt):
                st = S[g]
                if tt == 0:
                    vbd2 = p_vbd2.tile([128, 6 * G * 64], BF16, tag="vbd2",
                                       name="vbd2")
                    st["v4"] = vbd2.rearrange("p (j b c) -> p j b c",
                                              b=G, c=64)
                xT = st["xT"]
                to = 98 * tt
                scr = p_scr.tile([98, C], BF16, tag="scr", name="scr")
                for half in range(2):
                    pv = psS.tile([98, 384], F32, tag="psS", name="psS")
                    for ci in range(6):
                        nc.tensor.matmul(
                            pv[:], xT[:, ci, to:to + 98],
                            wq6[:, ci, 1536 + 384 * half:
                                1536 + 384 * (half + 1)],
                            start=(ci == 0), stop=(ci == 5))
                    nc.vector.tensor_copy(
                        scr[:, 384 * half:384 * (half + 1)], pv[:])
                sv = scr.rearrange("p (j two c) -> p j two c", two=2, c=64)
                v4 = st["v4"]
                for loc, b in ((0, 2 * tt), (49, 2 * tt + 1)):
                    nc.sync.dma_start(v4[0:49, :, b, :],
                                      sv[loc:loc + 49, :, 0, :])
                    nc.sync.dma_start(v4[64:113, :, b, :],
                                      sv[loc:loc + 49, :, 1, :])

            def d_head(g, j):
                st = S[g]
                if j == 0:
                    st["unT"] = [p_unT.tile([128, T], BF16, tag=f"unT{ci}",
                                            name=f"unT{ci}")
                                 for ci in range(6)]
                    st["stash"] = {}
                q, k = st["q"], st["k"]
                eT = eTs[j % 2]
                ps = psS.tile([128, T], F32, tag="psS", name="psS")
                if g == 0 and j < 2:
                    # first-ever uses of the psS slots: make the dead band
                    # finite so the single exp op below never sees raw
                    # uninitialized psum (32-aligned access requirement)
                    nc.vector.memset(ps[32:64, :], 0.0)
                for b in range(G):
                    bs = slice(49 * b, 49 * b + 49)
                    nc.tensor.matmul(ps[0:49, bs], k[j][0:64, bs],
                                     q[j][0:64, bs], start=True, stop=True)
                    nc.tensor.matmul(ps[64:113, bs], k[j][64:128, bs],
                                     q[j][64:128, bs], start=True, stop=True,
                                     tile_position=(64, 64))
                # rows 49:64 hold stale-but-finite values; exp of them is
                # multiplied by onesbd zeros in the row-sum
                nc.scalar.activation(eT[0:113, :], ps[0:113, :], EXP,
                                     scale=0.125)
                st["stash"][j] = [eT]

            def d_tail_a1(g, j):
                st = S[g]
                eT, = st["stash"][j]
                pv_bc = psV.tile([128, T], F32, tag="psV", name="psV")
                nc.tensor.matmul(pv_bc[0:2, :], onesbd[:], eT[:],
                                 start=True, stop=True)
                st["stash"][j] = [eT, pv_bc]

            def d_tail_a2(g, j):
                st = S[g]
                eT, pv_bc = st["stash"][j]
                rr = p_rr.tile([2, T], F32, tag="rr", name="rr")
                nc.vector.reciprocal_approx_fast(rr[:], pv_bc[0:2, :])
                rrb = p_rr.tile([2, T], BF16, tag="rrb", name="rrb")
                nc.vector.tensor_copy(rrb[:], rr[:])
                st["stash"][j] = [eT, pv_bc, rrb]

            def d_tail_b(g, j):
                st = S[g]
                eT, pv_bc, rrb = st["stash"][j]
                v4 = st["v4"]
                po = psO.tile([128, T], F32, tag="psO", name="psO")
                for b in range(G):
                    bs = slice(49 * b, 49 * b + 49)
                    nc.tensor.matmul(po[0:64, bs], v4[0:49, j, b, :],
                                     eT[0:49, bs], start=True, stop=True)
                    nc.tensor.matmul(po[64:128, bs], v4[64:113, j, b, :],
                                     eT[64:113, bs], start=True, stop=True,
                                     tile_position=(64, 64))
                st["stash"][j] = [pv_bc, rrb, po]

            def d_tail_c1(g, j):
                st = S[g]
                pv_bc, rrb, po = st["stash"][j]
                # broadcast 1/rowsum across the 2 head halves
                nc.tensor.matmul(pv_bc[:], selpair[:], rrb[:],
                                 start=True, stop=True)

            def d_tail_c2(g, j):
                st = S[g]
                pv_bc, rrb, po = st["stash"].pop(j)
                unT = st["unT"]
                nc.scalar.copy(unT[j][:], po[:])
                nc.vector.tensor_mul(out=unT[j][:], in0=unT[j][:],
                                     in1=pv_bc[:])

            def emit_E(g, j2):
                st = S[g]
                unT = st["unT"]
                pp = psO.tile([128, T], F32, tag="psO", name="psO")
                for ci in range(6):
                    nc.tensor.matmul(
                        pp[:], wp6[:, ci, 128 * j2:128 * (j2 + 1)],
                        unT[ci][:], start=(ci == 0), stop=(ci == 5))
                osb = p_osb.tile([128, T], BF16, tag="osb", name="osb")
                nc.scalar.add(osb[:], pp[:], bias_sb[:, j2:j2 + 1])
                nc.sync.dma_start(
                    d_out[128 * j2:128 * (j2 + 1), g * T:(g + 1) * T],
                    osb[:])

            # ---- bootstrap: group 0's B and C run standalone ----
            for jc in range(12):
                emit_B_chain(0, jc)
            load_xT(1)
            for tt in range(4):
                emit_C_unit(0, tt)

            # ---- steady state: head-pair double-steps. Each step first
            # closes the previous pair's tails (the two pr matmuls emitted
            # adjacent so they pipeline, likewise the two bc matmuls), then
            # runs the current pair's scores, then up to 2 E units of the
            # group whose attention just completed, then 4 B chains and up
            # to 2 C units of the next group. ----
            pending = None        # (g, j0, j1) pair awaiting its tails
            e_queue = []          # (g, j2) E units awaiting emission

            def tails_pre(pg, p0, p1):
                d_tail_a1(pg, p0)
                d_tail_a1(pg, p1)
                d_tail_a2(pg, p0)
                d_tail_a2(pg, p1)
                d_tail_b(pg, p0)
                d_tail_b(pg, p1)

            def tails_post(pg, p0, p1):
                d_tail_c1(pg, p0)
                d_tail_c1(pg, p1)
                d_tail_c2(pg, p0)
                d_tail_c2(pg, p1)
                if p1 == 5:
                    e_queue.extend((pg, j2) for j2 in range(6))

            for g in range(N_GROUPS):
                if g + 2 < N_GROUPS:
                    load_xT(g + 2)
                for jp in range(3):
                    held, pending = pending, None
                    if held:
                        tails_pre(*held)
                    d_head(g, 2 * jp)
                    d_head(g, 2 * jp + 1)
                    if held:
                        tails_post(*held)
                    pending = (g, 2 * jp, 2 * jp + 1)
                    for _ in range(2):
                        if e_queue:
                            emit_E(*e_queue.pop(0))
                    if g + 1 < N_GROUPS:
                        for c4 in range(4):
                            emit_B_chain(g + 1, 4 * jp + c4)
                        if jp >= 1:
                            emit_C_unit(g + 1, 2 * (jp - 1))
                            emit_C_unit(g + 1, 2 * jp - 1)

            # drain: close the final pair, then the remaining E units
            tails_pre(*pending)
            tails_post(*pending)
            for e in e_queue:
                emit_E(*e)

    nc.compile()
    return nc


def _prep_inputs(x, W_qkv, W_proj, b_proj):
    x = np.asarray(x, dtype=np.float32)
    B, N, Cc = x.shape
    consts = _consts()
    wqkv = np.ascontiguousarray(np.asarray(W_qkv, dtype=np.float32)).astype(BF)
    wproj = np.ascontiguousarray(np.asarray(W_proj, dtype=np.float32)).astype(BF)
    bias = np.ascontiguousarray(
        np.asarray(b_proj, dtype=np.float32).reshape(6, 128).T)
    x_bf = x.astype(BF)
    in_maps = []
    for i in range(NUM_CORES):
        xt = np.ascontiguousarray(
            x_bf[i * B_CORE:(i + 1) * B_CORE].reshape(TOK, Cc).T)
        m = {"x": xt, "wqkv": wqkv, "wproj": wproj, "bias": bias}
        m.update(consts)
        in_maps.append(m)
    return in_maps


def _unshard(results):
    out = np.empty((NUM_CORES * B_CORE, SEQ, C), dtype=np.float32)
    for i in range(NUM_CORES):
        o = np.asarray(results[i]["out"]).astype(np.float32)  # [C, TOK]
        out[i * B_CORE:(i + 1) * B_CORE] = o.T.reshape(B_CORE, SEQ, C)
    return out


def kernel(x, W_qkv, W_proj, b_proj):
    from concourse.bass_utils import run_bass_kernel_spmd

    if "nc" not in _CACHE:
        _CACHE["nc"] = _build()
    nc = _CACHE["nc"]

    in_maps = _prep_inputs(x, W_qkv, W_proj, b_proj)
    res = run_bass_kernel_spmd(nc, in_maps, list(range(NUM_CORES)))
    return _unshard(res.results)


# revision 19
# speedup vs baseline: 1.0540x; 1.0540x over previous
"""Trainium2 Bass kernel: fused attention block (QKV proj -> MHA -> out proj).

Reference (per batch item b, NUM_HEADS=12, Dh=64):
    qkv = x @ W_qkv; q,k,v per head
    attn = softmax(q @ k^T / 8) @ v
    out  = concat_heads(attn) @ W_proj + b_proj

Sharding: data-parallel over batch across 8 NeuronCores (128 batch items
per core), weights replicated. One SPMD Bass program, per-core inputs.

Design (bf16, feature-major I/O, half-array head pairing, fully
software-pipelined):
  - Host pre-transposes x to feature-major [C, TOK] bf16 and transposes
    the feature-major bf16 output back; device never transposes.
  - Groups of G=8 batches (T=392 tokens). Per group:
    B: q,k co-tiles [128, T] feature-major = Wqkv_slice.T @ xT.
    C: v token-major in 4 tiles of 98 tokens, scattered by SBUF->SBUF
       DMA into per-(head-pair j, batch b) blocks vbd2 [128, 64]:
       rows 0:49 = even-head key positions, 64:113 = odd-head.
    D: per j: 8 batches x 2 concurrent half-array matmuls (even head in
       array quadrants (0:64)x(0:64), odd head at tile_position=(64,64))
       for scores and attn@V; single-op exp on ACT; row sums via ones
       matmul; normalization via reciprocal_approx_fast + broadcast
       matmul + DVE multiply.
    E: out co-tiles [128, T] = Wproj_slice.T @ unT + per-partition bias.
  - Steady state runs D in head-pair double-steps and interleaves E of
    the previous group plus B and C of the next group into them, so the
    tensor engine sees one dense instruction stream and the HAM clock
    gate stays at full rate.
"""
import sys

sys.path.insert(0, "/opt/trn_rl_repo")

import numpy as np
import ml_dtypes

NUM_CORES = 8
B_CORE = 128          # batch items per core
SEQ = 49              # tokens per batch item
C = 768               # channels
H = 12                # heads
G = 8                 # batch items per group
T = SEQ * G           # 392 tokens per group
TOK = B_CORE * SEQ    # 6272 tokens per core
N_GROUPS = B_CORE // G

BF = ml_dtypes.bfloat16

_CACHE = {}


def _consts():
    onesbd = np.zeros((128, 2), dtype=BF)
    onesbd[0:49, 0] = 1.0
    onesbd[64:113, 1] = 1.0
    selpair = np.zeros((2, 128), dtype=BF)
    selpair[0, 0:64] = 1.0
    selpair[1, 64:128] = 1.0
    return {"onesbd": onesbd, "selpair": selpair}


def _build():
    import concourse.bacc as bacc
    import concourse.mybir as mybir
    import concourse.tile as tile

    F32 = mybir.dt.float32
    BF16 = mybir.dt.bfloat16
    EXP = mybir.ActivationFunctionType.Exp

    nc = bacc.Bacc("TRN2", target_bir_lowering=False)

    d_x = nc.declare_dram_parameter("x", [C, TOK], BF16, isOutput=False)
    d_wqkv = nc.declare_dram_parameter("wqkv", [C, 3 * C], BF16, isOutput=False)
    d_wproj = nc.declare_dram_parameter("wproj", [C, C], BF16, isOutput=False)
    d_bias = nc.declare_dram_parameter("bias", [128, 6], F32, isOutput=False)
    d_onesbd = nc.declare_dram_parameter("onesbd", [128, 2], BF16,
                                         isOutput=False)
    d_selpair = nc.declare_dram_parameter("selpair", [2, 128], BF16,
                                          isOutput=False)
    d_out = nc.declare_dram_parameter("out", [C, TOK], BF16, isOutput=True)

    # DRAM views with the 6x128 channel-tile structure exposed, so one DMA
    # moves all 6 channel tiles of a slice
    x6 = d_x.rearrange("(c p) t -> p c t", p=128)
    wqkv6 = d_wqkv.rearrange("(c p) n -> p c n", p=128)
    wproj6 = d_wproj.rearrange("(c p) n -> p c n", p=128)

    with tile.TileContext(nc) as tc, \
         nc.allow_low_precision(reason="bf16 matmuls within 2e-2 tolerance"):
        with tc.tile_pool(name="wres", bufs=1) as wres, \
             tc.tile_pool(name="qk", bufs=2) as p_qk, \
             tc.tile_pool(name="scr", bufs=3) as p_scr, \
             tc.tile_pool(name="vbd2", bufs=3) as p_vbd2, \
             tc.tile_pool(name="rr", bufs=3) as p_rr, \
             tc.tile_pool(name="unT", bufs=2) as p_unT, \
             tc.tile_pool(name="osb", bufs=2) as p_osb, \
             tc.tile_pool(name="psA", bufs=2, space="PSUM") as psA, \
             tc.tile_pool(name="psS", bufs=2, space="PSUM") as psS, \
             tc.tile_pool(name="psO", bufs=2, space="PSUM") as psO, \
             tc.tile_pool(name="psV", bufs=2, space="PSUM") as psV:

            # ---- resident weights / constants ----
            # q,k weight columns first: the first B-stage matmuls need
            # only these plus the group-0 x tiles
            w_qkv = wres.tile([128, 6 * 3 * C], BF16, tag="wqkv", name="wqkv")
            wq6 = w_qkv.rearrange("p (c n) -> p c n", n=3 * C)
            # chunked so the startup transfers spread across DMA queues
            for o in range(0, 1536, 384):
                nc.sync.dma_start(wq6[:, :, o:o + 384],
                                  wqkv6[:, :, o:o + 384])

            S = [dict() for _ in range(N_GROUPS)]

            # x resident feature-major for the whole core: B slices it per
            # group, C slices it in clean 128-token tiles across group
            # boundaries (no runt tiles, full partition utilization)
            xTall = wres.tile([128, 6 * TOK], BF16, tag="xTall", name="xTall")
            xv = xTall.rearrange("p (c t) -> p c t", t=TOK)

            def load_xT(g, chunks=1):
                step = T // chunks
                for o in range(0, T, step):
                    nc.sync.dma_start(
                        xv[:, :, g * T + o:g * T + o + step],
                        x6[:, :, g * T + o:g * T + o + step])

            load_xT(0, chunks=4)
            for o in range(1536, 2304, 384):
                nc.sync.dma_start(wq6[:, :, o:o + 384],
                                  wqkv6[:, :, o:o + 384])
            w_proj = wres.tile([128, 6 * C], BF16, tag="wproj", name="wproj")
            wp6 = w_proj.rearrange("p (c n) -> p c n", n=C)
            for o in range(0, C, 384):
                nc.sync.dma_start(wp6[:, :, o:o + 384],
                                  wproj6[:, :, o:o + 384])
            onesbd = wres.tile([128, 2], BF16, tag="onesbd")
            nc.sync.dma_start(onesbd[:], d_onesbd[:])
            selpair = wres.tile([2, 128], BF16, tag="selpair")
            nc.sync.dma_start(selpair[:], d_selpair[:])
            bias_sb = wres.tile([128, 6], F32, tag="bias_sb")
            nc.sync.dma_start(bias_sb[:], d_bias[:])
            # exp tiles: dead bands (rows 49:64, 113:128) must stay zero
            eTs = []
            for nm in ("eTa", "eTb"):
                t = wres.tile([128, T], BF16, tag=nm, name=nm)
                nc.vector.memset(t[:], 0.0)
                eTs.append(t)

            # ---- emission helpers ----
            def emit_B_chain(g, jc):
                st = S[g]
                if jc == 0:
                    st["q"], st["k"] = [None] * 6, [None] * 6
                pq = psA.tile([128, T], F32, tag="psA", name="psA")
                for ci in range(6):
                    nc.tensor.matmul(
                        pq[:], wq6[:, ci, 128 * jc:128 * (jc + 1)],
                        xv[:, ci, g * T:(g + 1) * T],
                        start=(ci == 0), stop=(ci == 5))
                nm = f"q{jc}" if jc < 6 else f"k{jc - 6}"
                t = p_qk.tile([128, T], BF16, tag=nm, name=nm)
                if jc % 2 == 0:
                    nc.scalar.copy(t[:], pq[:])
                else:
                    nc.vector.tensor_copy(t[:], pq[:])
                if jc < 6:
                    st["q"][jc] = t
                else:
                    st["k"][jc - 6] = t

            c_next = [0]

            def emit_C_unit():
                t4 = c_next[0]
                c_next[0] += 1
                tok0 = 128 * t4
                scr = p_scr.tile([128, C], BF16, tag="scr", name="scr")
                for half in range(2):
                    pv = psS.tile([128, 384], F32, tag="psS", name="psS")
                    for ci in range(6):
                        nc.tensor.matmul(
                            pv[:], xv[:, ci, tok0:tok0 + 128],
                            wq6[:, ci, 1536 + 384 * half:
                                1536 + 384 * (half + 1)],
                            start=(ci == 0), stop=(ci == 5))
                    nc.vector.tensor_copy(
                        scr[:, 384 * half:384 * (half + 1)], pv[:])
                sv = scr.rearrange("p (j two c) -> p j two c", two=2, c=64)
                for b in range(tok0 // SEQ, min(B_CORE, (tok0 + 127) // SEQ + 1)):
                    lo = max(SEQ * b, tok0)
                    hi = min(SEQ * (b + 1), tok0 + 128)
                    if lo >= hi:
                        continue
                    gb, bb = b // G, b % G
                    if "v4" not in S[gb]:
                        vbd2 = p_vbd2.tile([128, 6 * G * 64], BF16,
                                           tag="vbd2", name="vbd2")
                        S[gb]["v4"] = vbd2.rearrange(
                            "p (j b c) -> p j b c", b=G, c=64)
                    v4 = S[gb]["v4"]
                    sl, sh = lo - SEQ * b, hi - SEQ * b
                    nc.sync.dma_start(v4[sl:sh, :, bb, :],
                                      sv[lo - tok0:hi - tok0, :, 0, :])
                    nc.sync.dma_start(v4[64 + sl:64 + sh, :, bb, :],
                                      sv[lo - tok0:hi - tok0, :, 1, :])

            def ensure_C(tok_thresh):
                while c_next[0] < TOK // 128 and 128 * c_next[0] < tok_thresh:
                    emit_C_unit()

            def d_head(g, j):
                st = S[g]
                if j == 0:
                    st["unT"] = [p_unT.tile([128, T], BF16, tag=f"unT{ci}",
                                            name=f"unT{ci}")
                                 for ci in range(6)]
                    st["stash"] = {}
                q, k = st["q"], st["k"]
                eT = eTs[j % 2]
                ps = psS.tile([128, T], F32, tag="psS", name="psS")
                if g == 0 and j < 2:
                    # first-ever uses of the psS slots: make the dead band
                    # finite so the single exp op below never sees raw
                    # uninitialized psum (32-aligned access requirement)
                    nc.vector.memset(ps[32:64, :], 0.0)
                for b in range(G):
                    bs = slice(49 * b, 49 * b + 49)
                    nc.tensor.matmul(ps[0:49, bs], k[j][0:64, bs],
                                     q[j][0:64, bs], start=True, stop=True)
                    nc.tensor.matmul(ps[64:113, bs], k[j][64:128, bs],
                                     q[j][64:128, bs], start=True, stop=True,
                                     tile_position=(64, 64))
                # rows 49:64 hold stale-but-finite values; exp of them is
                # multiplied by onesbd zeros in the row-sum
                nc.scalar.activation(eT[0:113, :], ps[0:113, :], EXP,
                                     scale=0.125)
                st["stash"][j] = [eT]

            def d_tail_a1(g, j):
                st = S[g]
                eT, = st["stash"][j]
                pv_bc = psV.tile([128, T], F32, tag="psV", name="psV")
                nc.tensor.matmul(pv_bc[0:2, :], onesbd[:], eT[:],
                                 start=True, stop=True)
                st["stash"][j] = [eT, pv_bc]

            def d_tail_a2(g, j):
                st = S[g]
                eT, pv_bc = st["stash"][j]
                rr = p_rr.tile([2, T], F32, tag="rr", name="rr")
                nc.vector.reciprocal_approx_fast(rr[:], pv_bc[0:2, :])
                rrb = p_rr.tile([2, T], BF16, tag="rrb", name="rrb")
                nc.vector.tensor_copy(rrb[:], rr[:])
                st["stash"][j] = [eT, pv_bc, rrb]

            def d_tail_b(g, j):
                st = S[g]
                eT, pv_bc, rrb = st["stash"][j]
                v4 = st["v4"]
                po = psO.tile([128, T], F32, tag="psO", name="psO")
                for b in range(G):
                    bs = slice(49 * b, 49 * b + 49)
                    nc.tensor.matmul(po[0:64, bs], v4[0:49, j, b, :],
                                     eT[0:49, bs], start=True, stop=True)
                    nc.tensor.matmul(po[64:128, bs], v4[64:113, j, b, :],
                                     eT[64:113, bs], start=True, stop=True,
                                     tile_position=(64, 64))
                st["stash"][j] = [pv_bc, rrb, po]

            def d_tail_c1(g, j):
                st = S[g]
                pv_bc, rrb, po = st["stash"][j]
                # broadcast 1/rowsum across the 2 head halves
                nc.tensor.matmul(pv_bc[:], selpair[:], rrb[:],
                                 start=True, stop=True)

            def d_tail_c2(g, j):
                st = S[g]
                pv_bc, rrb, po = st["stash"].pop(j)
                unT = st["unT"]
                nc.scalar.copy(unT[j][:], po[:])
                nc.vector.tensor_mul(out=unT[j][:], in0=unT[j][:],
                                     in1=pv_bc[:])

            def emit_E(g, j2):
                st = S[g]
                unT = st["unT"]
                pp = psO.tile([128, T], F32, tag="psO", name="psO")
                for ci in range(6):
                    nc.tensor.matmul(
                        pp[:], wp6[:, ci, 128 * j2:128 * (j2 + 1)],
                        unT[ci][:], start=(ci == 0), stop=(ci == 5))
                osb = p_osb.tile([128, T], BF16, tag="osb", name="osb")
                nc.scalar.add(osb[:], pp[:], bias_sb[:, j2:j2 + 1])
                nc.sync.dma_start(
                    d_out[128 * j2:128 * (j2 + 1), g * T:(g + 1) * T],
                    osb[:])

            # ---- bootstrap: group 0's B and C run standalone ----
            for jc in range(12):
                emit_B_chain(0, jc)
            load_xT(1)
            ensure_C(T)

            # ---- steady state: head-pair double-steps. Each step first
            # closes the previous pair's tails (the two pr matmuls emitted
            # adjacent so they pipeline, likewise the two bc matmuls), then
            # runs the current pair's scores, then up to 2 E units of the
            # group whose attention just completed, then 4 B chains and up
            # to 2 C units of the next group. ----
            pending = None        # (g, j0, j1) pair awaiting its tails
            e_queue = []          # (g, j2) E units awaiting emission

            def tails_pre(pg, p0, p1):
                d_tail_a1(pg, p0)
                d_tail_a1(pg, p1)
                d_tail_a2(pg, p0)
                d_tail_a2(pg, p1)
                d_tail_b(pg, p0)
                d_tail_b(pg, p1)

            def tails_post(pg, p0, p1):
                d_tail_c1(pg, p0)
                d_tail_c1(pg, p1)
                d_tail_c2(pg, p0)
                d_tail_c2(pg, p1)
                if p1 == 5:
                    e_queue.extend((pg, j2) for j2 in range(6))

            for g in range(N_GROUPS):
                if g + 2 < N_GROUPS:
                    load_xT(g + 2)
                for jp in range(3):
                    held, pending = pending, None
                    if held:
                        tails_pre(*held)
                    d_head(g, 2 * jp)
                    d_head(g, 2 * jp + 1)
                    if held:
                        tails_post(*held)
                    pending = (g, 2 * jp, 2 * jp + 1)
                    for _ in range(2):
                        if e_queue:
                            emit_E(*e_queue.pop(0))
                    if g + 1 < N_GROUPS:
                        for c4 in range(4):
                            emit_B_chain(g + 1, 4 * jp + c4)
                        ensure_C((g + 1) * T + (jp + 1) * T // 3)

            # drain: close the final pair, then the remaining E units
            tails_pre(*pending)
            tails_post(*pending)
            for e in e_queue:
                emit_E(*e)

    nc.compile()
    return nc


def _prep_inputs(x, W_qkv, W_proj, b_proj):
    x = np.asarray(x, dtype=np.float32)
    B, N, Cc = x.shape
    consts = _consts()
    wqkv = np.ascontiguousarray(np.asarray(W_qkv, dtype=np.float32)).astype(BF)
    wproj = np.ascontiguousarray(np.asarray(W_proj, dtype=np.float32)).astype(BF)
    bias = np.ascontiguousarray(
        np.asarray(b_proj, dtype=np.float32).reshape(6, 128).T)
    x_bf = x.astype(BF)
    in_maps = []
    for i in range(NUM_CORES):
        xt = np.ascontiguousarray(
            x_bf[i * B_CORE:(i + 1) * B_CORE].reshape(TOK, Cc).T)
        m = {"x": xt, "wqkv": wqkv, "wproj": wproj, "bias": bias}
        m.update(consts)
        in_maps.append(m)
    return in_maps


def _unshard(results):
    out = np.empty((NUM_CORES * B_CORE, SEQ, C), dtype=np.float32)
    for i in range(NUM_CORES):
        o = np.asarray(results[i]["out"]).astype(np.float32)  # [C, TOK]
        out[i * B_CORE:(i + 1) * B_CORE] = o.T.reshape(B_CORE, SEQ, C)
    return out


def kernel(x, W_qkv, W_proj, b_proj):
    from concourse.bass_utils import run_bass_kernel_spmd

    if "nc" not in _CACHE:
        _CACHE["nc"] = _build()
    nc = _CACHE["nc"]

    in_maps = _prep_inputs(x, W_qkv, W_proj, b_proj)
    res = run_bass_kernel_spmd(nc, in_maps, list(range(NUM_CORES)))
    return _unshard(res.results)
